# revision 1
# baseline (speedup 1.0000x reference)
"""Trainium2 Bass kernel for the CFM toy model (pairwise-force GNN).

Strategy (per core, 8 cores, row-parallel over i):
  - Each core owns R=256 rows i; all j (2048) are local.
  - Plane phase: compute dx_norm_x / dx_norm_y / distance planes in a
    [128 rows, 2048 j] layout with fully-packed DVE/ACT ops, emitted as
    j-sliced chains so the serial latency pipelines.
  - Token phase: regroup 4 rows x 2048 j into a [20, 2048] feature tile
    (partition = feature k * 4 + group g) via one SBUF->SBUF DMA, then run
    the tiny 5->32->32 MLP as block-diagonal float32r matmuls over 4
    row-groups at once.  ReLU+bias(layer1) on DVE; ReLU+bias(layer2) on ACT
    with accum_out emitting the per-row running sum, so layer 3 (linear) is
    applied once per row on the summed hidden state instead of per pair.
  - Diagonal (j==i) forces and the base-velocity MLP are computed in small
    side passes; output = bv + (sum_j - diag) / (N-1) + biases.
"""

import numpy as np
import concourse.bass as bass
from concourse import mybir
from concourse.tile import TileContext
from bass_rust import SyncInfo

F32 = mybir.dt.float32
F32R = mybir.dt.float32r
AF = mybir.ActivationFunctionType
OP = mybir.AluOpType

N = 2048
NCORES = 8
R = N // NCORES          # 256 rows per core
NBLK = R // 128          # 2 blocks of 128 rows
CHUNKS = R // 4          # 64 chunks of 4 rows
EPS = 1e-6

# wpack column layout (f32r block-diagonal weights)
_WP = dict(w1=(20, 0, 128), w2=(128, 128, 256))
WPACK_COLS = 256
# bpack column layout (fp32 biases + per-partition scalars)
_BP = dict(b1=0, b2=1, pxi0=2, pxi1=3, pyi0=4, pyi1=5)
BPACK_COLS = 6


def split_sync_waits(nc, max_waits=1):
    """This walrus build rejects instructions carrying more than one sync
    wait; move extra waits onto nops inserted before the instruction (same
    engine, same basic block => same sequencer order)."""
    for f in nc.m.functions:
        for bb in f.blocks:
            insts = bb.instructions
            out = []
            changed = False
            for inst in insts:
                si = getattr(inst, "sync_info", None)
                waits = list(si.on_wait) if si is not None else []
                if len(waits) > max_waits:
                    changed = True
                    extra = waits[:-max_waits]
                    keep = waits[-max_waits:]
                    for i in range(0, len(extra), max_waits):
                        nop = mybir.InstNoOp(
                            name=nc.get_next_instruction_name(),
                            ins=[], outs=[])
                        nop.engine = inst.engine
                        nop.sync_info = SyncInfo(
                            on_wait=extra[i:i + max_waits], on_update=[])
                        out.append(nop)
                    inst.sync_info = SyncInfo(
                        on_wait=keep, on_update=list(si.on_update))
                out.append(inst)
            if changed:
                bb.instructions = out


def build_program(split_waits=True):
    nc = bass.Bass("TRN2", target_bir_lowering=False, debug=False,
                   num_devices=NCORES)

    xyj = nc.dram_tensor("xyj", [2, N], F32, kind="ExternalInput").ap()
    constsr = nc.dram_tensor("constsr", [2, 4 * N], F32R,
                             kind="ExternalInput").ap()
    wpackd = nc.dram_tensor("wpack", [128, WPACK_COLS], F32R,
                            kind="ExternalInput").ap()
    bpackd = nc.dram_tensor("bpack", [128, BPACK_COLS], F32,
                            kind="ExternalInput").ap()
    y = nc.dram_tensor("y", [128, 2 * CHUNKS], F32,
                       kind="ExternalOutput").ap()

    with TileContext(nc) as tc:
        with (
            tc.tile_pool(name="const", bufs=1) as cpool,
            tc.tile_pool(name="planes", bufs=1) as fpool,
            tc.tile_pool(name="work", bufs=1) as wpool,
            tc.tile_pool(name="psum", bufs=1, space="PSUM") as ppool,
        ):
            # ---- setup DMAs (ordered so the plane phase can start ASAP) --
            PXJ = cpool.tile([128, N], F32, tag="pxj")
            PYJ = cpool.tile([128, N], F32, tag="pyj")
            bpack = cpool.tile([128, BPACK_COLS], F32, tag="bpack")
            wpack = cpool.tile([128, WPACK_COLS], F32R, tag="wpack")
            nc.sync.dma_start(bpack[:, :], bpackd[:, :])
            HN2 = N // 2
            for h in range(2):
                hsl = slice(h * HN2, (h + 1) * HN2)
                nc.sync.dma_start(PXJ[:, hsl],
                                  xyj[0:1, hsl].to_broadcast((128, HN2)))
                nc.sync.dma_start(PYJ[:, hsl],
                                  xyj[1:2, hsl].to_broadcast((128, HN2)))
            nc.sync.dma_start(wpack[:, :], wpackd[:, :])

            def wv(key):
                p, c0, c1 = _WP[key]
                return wpack[0:p, c0:c1]

            def bv_(key, p=128):
                c = _BP[key]
                return bpack[0:p, c:c + 1]

            w1s, w2s = wv("w1"), wv("w2")
            b1s, b2s = bv_("b1"), bv_("b2")
            pxi = [bv_("pxi0"), bv_("pxi1")]
            pyi = [bv_("pyi0"), bv_("pyi1")]

            # ---- PSUM tensors: 4 x [128,1024] = 8 banks ----
            PA = [ppool.tile([128, 1024], F32, tag=f"pa{i}", name=f"pa{i}")
                  for i in range(2)]
            PB = [ppool.tile([128, 1024], F32, tag=f"pb{i}", name=f"pb{i}")
                  for i in range(2)]

            # ---- plane phase: F[b] = [dxnx | dxny | d], each [128, N] ----
            # Emitted as two j-halves per block, interleaved, so the serial
            # op chain pipelines across slices instead of adding latency.
            F = [fpool.tile([128, 3 * N], F32R, tag=f"F{b}", name=f"F{b}")
                 for b in range(NBLK)]
            DXX = fpool.tile([128, N], F32, tag="dxx")
            DXY = fpool.tile([128, N], F32, tag="dxy")
            SQ = fpool.tile([128, N], F32, tag="sq")
            RP = fpool.tile([128, N], F32, tag="rp")

            def emit_planes(b):
                Fb = F[b]
                HN = N // 2
                sls = [slice(0, HN), slice(HN, N)]
                for sl in sls:
                    nc.vector.tensor_scalar(DXX[:, sl], PXJ[:, sl],
                                            pxi[b], -1.0,
                                            op0=OP.subtract, op1=OP.mult)
                    nc.vector.tensor_scalar(DXY[:, sl], PYJ[:, sl],
                                            pyi[b], -1.0,
                                            op0=OP.subtract, op1=OP.mult)
                for sl in sls:
                    nc.scalar.activation(SQ[:, sl], DXX[:, sl], AF.Square)
                    nc.scalar.activation(RP[:, sl], DXY[:, sl], AF.Square)
                for sl in sls:
                    nc.vector.tensor_tensor(SQ[:, sl], SQ[:, sl], RP[:, sl],
                                            op=OP.add)
                for sl in sls:
                    dsl = slice(2 * N + sl.start, 2 * N + sl.stop)
                    nc.scalar.activation(Fb[:, dsl], SQ[:, sl], AF.Sqrt)
                for sl in sls:
                    dsl = slice(2 * N + sl.start, 2 * N + sl.stop)
                    nc.vector.tensor_scalar(SQ[:, sl],
                                            Fb[:, dsl].bitcast(F32), EPS,
                                            None, op0=OP.add)
                    nc.vector.reciprocal(RP[:, sl], SQ[:, sl])
                for sl in sls:
                    xsl = slice(N + sl.start, N + sl.stop)
                    nc.vector.tensor_tensor(Fb[:, sl.start:sl.stop],
                                            DXX[:, sl], RP[:, sl],
                                            op=OP.mult)
                    nc.vector.tensor_tensor(Fb[:, xsl], DXY[:, sl],
                                            RP[:, sl], op=OP.mult)

            emit_planes(0)

            # ---- token supertiles: 4 chunks each, triple-buffered ----
            T = [wpool.tile([20, 4 * N], F32R, tag=f"T{i}", name=f"T{i}")
                 for i in range(3)]
            for Ti in T:
                nc.sync.dma_start(Ti[12:16, :],
                                  constsr[0:1, :].to_broadcast((4, 4 * N)))
                nc.sync.dma_start(Ti[16:20, :],
                                  constsr[1:2, :].to_broadcast((4, 4 * N)))
            H = [wpool.tile([128, 1024], F32R, tag=f"H{i}", name=f"H{i}")
                 for i in range(2)]
            Sa = wpool.tile([128, 1024], F32, tag="sa")
            Hs = wpool.tile([128, 2 * CHUNKS], F32, tag="hs")

            emit_planes(1)

            # ---- main loop: 64 chunks x 2 j-halves, software-pipelined ----
            # Emit L1 of iteration t+1 before L2 of iteration t so the
            # in-order PE always has an independent matmul to run while the
            # DVE relu of iteration t is in flight.
            NSUP = CHUNKS // 4          # 16 supertiles of 4 chunks

            # F rows are stored g-major (partition p holds block-row
            # 4*(p%32) + p//32, arranged via the host-permuted pxi/pyi
            # scalars), so each (feature, group) supertile read is a
            # partition-contiguous 4-row slice.
            def emit_tdma(sp):
                b = sp // 8                 # block (8 supers per block)
                sl = sp % 8                 # super index within block
                Tt = T[sp % 3]
                for f in range(3):
                    eng = nc.gpsimd if f == 2 else nc.sync
                    for g in range(4):
                        r0 = g * 32 + 4 * sl
                        src = F[b][r0:r0 + 4, f * N:(f + 1) * N]
                        dst = Tt[4 * f + g:4 * f + g + 1, :]
                        eng.dma_start(dst, src)

            def emit_l1(t):
                c, jh = t // 2, t % 2
                Tt, pa = T[(c // 4) % 3], PA[t % 2]
                j0 = (c % 4) * N + jh * 1024
                for s in range(2):
                    nc.tensor.matmul(pa[:, s * 512:(s + 1) * 512],
                                     w1s,
                                     Tt[:, j0 + s * 512:j0 + (s + 1) * 512],
                                     start=True, stop=True)

            emit_tdma(0)
            emit_tdma(1)
            emit_l1(0)
            for t in range(2 * CHUNKS):
                if t % 8 == 0 and t // 8 + 2 < NSUP:
                    emit_tdma(t // 8 + 2)
                pa, pb, ht = PA[t % 2], PB[t % 2], H[t % 2]
                nc.vector.tensor_scalar(ht[:, :], pa[:, :], b1s, 0.0,
                                        op0=OP.add, op1=OP.max)

                def emit_l2():
                    for s in range(2):
                        nc.tensor.matmul(pb[:, s * 512:(s + 1) * 512],
                                         w2s,
                                         ht[:, s * 512:(s + 1) * 512],
                                         start=True, stop=True)

                if t + 1 < 2 * CHUNKS:
                    emit_l1(t + 1)
                emit_l2()
                nc.scalar.activation(Sa[:, :], pb[:, :], AF.Relu,
                                     bias=b2s,
                                     accum_out=Hs[:, t:t + 1])

            nc.sync.dma_start(y[:, :], Hs[:, :])

    if split_waits:
        split_sync_waits(nc)
    return nc


def make_in_maps(x, iW1, iW2):
    """Host-side prep: transposes + block-diagonal weight packing."""
    f = np.float32
    x = np.asarray(x, f)
    ib1v = None  # biases folded on device via bpack

    wpack = np.zeros((128, WPACK_COLS), f)
    w1blk = np.zeros((20, 128), f)
    for k in range(5):
        for g in range(4):
            w1blk[k * 4 + g, g * 32:(g + 1) * 32] = iW1[k]
    w2blk = np.zeros((128, 128), f)
    for g in range(4):
        w2blk[g * 32:(g + 1) * 32, g * 32:(g + 1) * 32] = iW2
    p, c0, c1 = _WP["w1"]
    wpack[0:p, c0:c1] = w1blk
    p, c0, c1 = _WP["w2"]
    wpack[0:p, c0:c1] = w2blk

    common = dict(
        wpack=wpack,
        xyj=np.ascontiguousarray(x.T[0:2]),
        constsr=np.ascontiguousarray(
            np.stack([np.ones(4 * N, f), np.tile(x[:, 2].astype(f), 4)])),
    )
    return common, x


def finalize_maps(common, x, ib1v, ib2v):
    f = np.float32
    maps = []
    for c in range(NCORES):
        xr = x[c * R:(c + 1) * R, :]
        bp = np.zeros((128, BPACK_COLS), f)
        bp[:, _BP["b1"]] = np.tile(np.asarray(ib1v, f), 4)
        bp[:, _BP["b2"]] = np.tile(np.asarray(ib2v, f), 4)
        perm = (4 * (np.arange(128) % 32) + np.arange(128) // 32)
        bp[:, _BP["pxi0"]] = xr[perm, 0]
        bp[:, _BP["pxi1"]] = xr[128 + perm, 0]
        bp[:, _BP["pyi0"]] = xr[perm, 1]
        bp[:, _BP["pyi1"]] = xr[128 + perm, 1]
        m = dict(common)
        m["bpack"] = bp
        maps.append(m)
    return maps


def _mlp_np(h, W1, b1, W2, b2, W3, b3):
    f = np.float32
    h = np.maximum(h @ np.asarray(W1, f) + np.asarray(b1, f), 0.0)
    h = np.maximum(h @ np.asarray(W2, f) + np.asarray(b2, f), 0.0)
    out = h @ np.asarray(W3, f)
    if b3 is not None:
        out = out + np.asarray(b3, f)
    return out


def assemble_output(results, x, iW3, ib3v, bW1, bb1v, bW2, bb2v, bW3, bb3v,
                    iW1, ib1v, iW2, ib2v):
    """Host epilogue: W3 application + diagonal correction + base velocity.
    All O(N) work on tiny matrices; the O(N^2) pair sums come from the
    device as per-row summed hidden states."""
    f = np.float32
    x = np.asarray(x, f)
    iW3 = np.asarray(iW3, f)
    out = np.empty((N, 2), f)
    # diagonal features: (0, 0, 0, 1, t_i); fii excludes b3
    feat_ii = np.zeros((N, 5), f)
    feat_ii[:, 3] = 1.0
    feat_ii[:, 4] = x[:, 2]
    fii = _mlp_np(feat_ii, iW1, ib1v, iW2, ib2v, iW3, None)
    bv = _mlp_np(x, bW1, bb1v, bW2, bb2v, bW3, bb3v)
    cfin = np.asarray(ib3v, f)
    for c in range(NCORES):
        hs = np.asarray(results[c]["y"], f)          # [128, 2*CHUNKS]
        hfin = hs[:, 0::2] + hs[:, 1::2]             # [128, CHUNKS]
        fsum = np.empty((R, 2), f)
        for g in range(4):
            fg = hfin[g * 32:(g + 1) * 32, :].T @ iW3   # [CHUNKS, 2]
            fsum[g::4, :] = fg
        rows = slice(c * R, (c + 1) * R)
        out[rows] = bv[rows] + cfin + (fsum - fii[rows]) / f(N - 1)
    return out


# ---------------------------------------------------------------------------
# Harness entry point
# ---------------------------------------------------------------------------
_CACHED_NC = None


def _get_nc():
    global _CACHED_NC
    if _CACHED_NC is None:
        _CACHED_NC = build_program()
    return _CACHED_NC


def _run(in_maps, **kw):
    from concourse.bass_utils import run_bass_kernel_spmd
    nc = _get_nc()
    return run_bass_kernel_spmd(nc, in_maps, list(range(NCORES)), **kw)


def kernel(**inputs):
    f = np.float32
    x = np.asarray(inputs["x"], f)
    common, xf = make_in_maps(x, np.asarray(inputs["iW1"], f),
                              np.asarray(inputs["iW2"], f))
    maps = finalize_maps(common, xf, np.asarray(inputs["ib1"], f),
                         np.asarray(inputs["ib2"], f))
    res = _run(maps)
    return assemble_output(
        res.results, xf,
        np.asarray(inputs["iW3"], f), np.asarray(inputs["ib3"], f),
        np.asarray(inputs["bW1"], f), np.asarray(inputs["bb1"], f),
        np.asarray(inputs["bW2"], f), np.asarray(inputs["bb2"], f),
        np.asarray(inputs["bW3"], f), np.asarray(inputs["bb3"], f),
        np.asarray(inputs["iW1"], f), np.asarray(inputs["ib1"], f),
        np.asarray(inputs["iW2"], f), np.asarray(inputs["ib2"], f))

